# revision 39
# baseline (speedup 1.0000x reference)
"""Multi-head attention (B=1, L=4096, C=512, H=8, D=64) on 8 TRN2 NeuronCores.

Sharding: head-parallel — core h computes head h end-to-end (QKV projection
for its head, softmax attention, and its partial contribution to the output
projection). Host sums the 8 partial output projections and adds the bias.

v3 structure (per core): a single software-pipelined stream keyed on
"items" k = (query-slice isl, key-tile-pair g):
  - ScalarE exp is the kernel bottleneck (16.8M scores at 1 elem/lane/cycle
    @1.2GHz -> ~110us floor); everything else is scheduled to keep the
    Activation queue back-to-back.
  - per item: ACT(k) -> emit scores(k+2) (stp double-buffer just freed by
    ACT(k)) -> PV(k). PE fills ACT wait time; the FIFO never blocks ACT.
  - stage 1 (qkT/kqT) and stage 2 (v tiles) interleave into slice 0's
    items as their xt DMA chunks (16x [128,1024], contiguous rows) land.
  - out-projection for slice isl is deferred a few items so the softmax
    denominator's DRAM transpose-bounce latency is hidden.
  - scores run PAIRED in disjoint PE row-groups (K=64 each, tile_position
    derived from base partitions 0/64); PV pairs accumulate into the two
    PSUM bank halves of pvp.
"""

import numpy as np
import ml_dtypes

L, C, D, H = 4096, 512, 64, 8
N_CORES = 8
P = 128

_BF16 = ml_dtypes.bfloat16

# correction h(v) = C2 + (v-1)(C1 + C0 v) ~ 2^(v-1)/v on [1,2)  (3.5e-3 minimax)
_EXP_C0, _EXP_C1, _EXP_C2 = 0.233658037, -0.460346027, 0.996513065
_EXP_SCALE, _EXP_BIAS = 8388608.0, 1065353216.0  # 2^23, 127*2^23

_exp_ops = {}


def _register_exp_ops():
    """Register two custom DVE ops implementing exp2 via the int-convert
    write path (Schraudolph) + a mantissa-polynomial correction:
      EXP2_INT_ANT:  i32 out = convert(z*2^23 + 127*2^23)   [exact: value
                     is integral in fp32, so round/trunc agree]
      EXP2_CORR_ANT: in = that tile bitcast to f32 = 2^floor(z)*(1+frac);
                     v = 1+frac via mantissa mask; out = in * h(v), bf16.
    """
    if _exp_ops:
        return
    from concourse import dve_ops
    from concourse.dve_spec import (Spec, Src0, C0, C1, C2, Zero, One,
                                    Bin, AluOp)

    def _int_ref(in0, in1, s0, s1, imm2):
        return np.rint(in0.astype(np.float64) * s0 + s1).astype(np.float32)

    def _corr_ref(in0, in1, s0, s1, imm2):
        n = np.ascontiguousarray(in0, dtype=np.float32).view(np.int32)
        v = ((n & np.int32(0x007FFFFF)) | np.int32(0x3F800000)).view(np.float32)
        h = (np.float32(imm2) + (v - np.float32(1.0)) *
             (np.float32(s1) + np.float32(s0) * v)).astype(np.float32)
        return (in0 * h).astype(np.float32)

    int_spec = Spec(body=Src0 * C0 + C1, reference=_int_ref)

    # mantissa mask 0x007FFFFF arrives bit-exact via the C3 (Src1-spill)
    # scalar slot — the value is an fp32 denormal, shipped from the host
    from concourse.dve_spec import C3, _spill_c3_to_src1
    q = Bin(AluOp.BITWISE_AND, Src0, C3)
    v = Bin(AluOp.BITWISE_OR, q, One)
    h = C2 + (v - One) * (C1 + C0 * v)
    corr_spec = Spec(body=_spill_c3_to_src1(Src0 * h), reference=_corr_ref)

    for name, spec in (("EXP2_INT_ANT", int_spec), ("EXP2_CORR_ANT", corr_spec)):
        op = dve_ops.DveOp(name, spec, subdim=False, uops_sha={})
        dve_ops.OPS.append(op)
        dve_ops.CUSTOM_DVE_SPECS[name] = spec
        dve_ops._SUB_OPCODE_FOR_NAME[name] = (
            dve_ops._CUSTOM_DVE_ROW_BASE + len(dve_ops.OPS) - 1
        )
        # pin the sha this build produces (no golden available out-of-tree)
        for ver in ("v3", "v4"):
            try:
                op.compile(ver)
            except ValueError as exc:
                got = str(exc).split("≠")[0].split(":")[-1].strip().strip("'\" ")
                op.uops_sha[ver] = got
                dve_ops._COMPILE_CACHE.pop((name, ver), None)
                op.compile(ver)
        _exp_ops[name] = op


def build_nc(L=L, C=C, D=D, reps=1, ablate=(), e_bufs=3, st_bufs=2,
             pv_bufs=1, proj_defer=3, dve_g="", dve_lead=1, cal=0, ni=1):
    import contextlib
    import concourse.bacc as bacc
    import concourse.mybir as mybir
    import concourse.tile as tile

    if cal:
        # calibration build: trivial loop body to measure For_i back-edge cost
        nc = bacc.Bacc("TRN2", target_bir_lowering=False, debug=False)
        y_d = nc.dram_tensor("y", [L, C], mybir.dt.float32, kind="ExternalOutput")
        xt_d = nc.dram_tensor("xt", [4, C, 1024], mybir.dt.bfloat16,
                              kind="ExternalInput")
        wqk_d = nc.dram_tensor("wqk", [C, P], mybir.dt.bfloat16, kind="ExternalInput")
        wv_d = nc.dram_tensor("wv", [C, D], mybir.dt.bfloat16, kind="ExternalInput")
        wo_d = nc.dram_tensor("wo", [D, C], mybir.dt.bfloat16, kind="ExternalInput")
        cst_d = nc.dram_tensor("cst", [P, 1], mybir.dt.float32, kind="ExternalInput")
        with tile.TileContext(nc) as tc:
            with tc.tile_pool(name="c", bufs=1) as cp:
                t = cp.tile([P, 512], mybir.dt.float32, name="t", tag="t")
                rep_ctx = tc.For_i(0, reps, 1) if reps > 1 else contextlib.nullcontext()
                with rep_ctx:
                    nc.vector.memset(t[:], 1.0)
                nc.sync.dma_start(y_d[:P, :], t[:])
        nc.compile()
        return nc

    _register_exp_ops()
    dve_groups = set(int(x) for x in str(dve_g).split(",") if x != "")

    f32 = mybir.dt.float32
    bf16 = mybir.dt.bfloat16
    i32 = mybir.dt.int32
    Exp = mybir.ActivationFunctionType.Exp
    Copy = mybir.ActivationFunctionType.Copy
    LN2 = 0.6931471805599453

    CT = C // P          # contraction tiles over channels (4)
    LT = L // P          # key tiles (32)
    NSL = L // 512       # 512-wide l-slices (8)
    NPAIR = LT // 2      # key tile pairs per slice (16)
    LPC = 4              # xt column chunks (1024 cols each)

    nc = bacc.Bacc("TRN2", target_bir_lowering=False, debug=False)

    xt_d = nc.dram_tensor("xt", [LPC, C, 1024], bf16, kind="ExternalInput")
    wqk_d = nc.dram_tensor("wqk", [C, P], bf16, kind="ExternalInput")
    wv_d = nc.dram_tensor("wv", [C, D], bf16, kind="ExternalInput")
    wo_d = nc.dram_tensor("wo", [D, C], bf16, kind="ExternalInput")
    cst_d = nc.dram_tensor("cst", [P, 1], f32, kind="ExternalInput")
    y_d = nc.dram_tensor("y", [L, C], f32, kind="ExternalOutput")

    with tile.TileContext(nc) as tc:
        with (
            tc.tile_pool(name="const", bufs=1) as constp,
            tc.tile_pool(name="xtp", bufs=1) as xtp,
            tc.tile_pool(name="qkv", bufs=1) as qkvp,
            tc.tile_pool(name="exps", bufs=e_bufs) as expp,
            tc.tile_pool(name="e0p", bufs=2) as e0p,
            tc.tile_pool(name="aon", bufs=2) as aop,
            tc.tile_pool(name="rowp", bufs=2) as rowp,
            tc.tile_pool(name="yp", bufs=4) as yp,
            tc.tile_pool(name="drs", bufs=2, space="DRAM") as drsp,
            tc.tile_pool(name="st_ps", bufs=st_bufs, space="PSUM") as stps,
            tc.tile_pool(name="pv_ps", bufs=pv_bufs, space="PSUM") as pvps,
            tc.tile_pool(name="pp_ps", bufs=2, space="PSUM") as pps,
        ):
            # ---- weight/input tiles; DMAs are emitted on demand so the
            # first stage-1 chunk isn't queued behind cold data
            wqk_sb = constp.tile([P, CT, P], bf16, name="wqk_sb", tag="wqk")
            wv_sb = constp.tile([P, CT, D], bf16, name="wv_sb", tag="wv")
            wo_sb = constp.tile([D, C], bf16, name="wo_sb", tag="wo")
            cst_sb = constp.tile([P, 1], f32, name="cst_sb", tag="cst")
            xs = [[None] * CT for _ in range(LPC)]
            for lp in range(LPC):
                for ct in range(CT):
                    xs[lp][ct] = xtp.tile([P, 1024], bf16, name=f"xt{lp}_{ct}",
                                          tag=f"xt{lp}_{ct}")

            qkT = qkvp.tile([P, L], bf16, name="qkT", tag="qkT")
            kqT = qkvp.tile([P, L], bf16, name="kqT", tag="kqT")
            v_sb = qkvp.tile([P, LT, D + 1], bf16, name="v_sb", tag="v")
            ao_all = qkvp.tile([D, L], bf16, name="ao_all", tag="ao_all")
            rec_all = qkvp.tile([P, LT], f32, name="rec_all", tag="rec_all")

            def one_body():
              nc.vector.memset(v_sb[:, :, D], 1.0)

              dma_done = set()

              def emit_wqk():
                  if "wqk" in dma_done:
                      return
                  dma_done.add("wqk")
                  nc.sync.dma_start(cst_sb[:], cst_d[:])
                  for ct in range(CT):
                      nc.sync.dma_start(wqk_sb[:, ct, :],
                                        wqk_d[ct * P : (ct + 1) * P, :])

              def emit_wv():
                  if "wv" in dma_done:
                      return
                  dma_done.add("wv")
                  for ct in range(CT):
                      nc.sync.dma_start(wv_sb[:, ct, :],
                                        wv_d[ct * P : (ct + 1) * P, :])

              def emit_wo():
                  if "wo" in dma_done:
                      return
                  dma_done.add("wo")
                  nc.sync.dma_start(wo_sb[:], wo_d[:])

              def emit_chunk(lp_hi):
                  for lp in range(lp_hi + 1):
                      if lp in dma_done:
                          continue
                      dma_done.add(lp)
                      for ct in range(CT):
                          nc.sync.dma_start(xs[lp][ct][:],
                                            xt_d[lp, ct * P : (ct + 1) * P, :])

              stage1_done = [False] * NSL

              def emit_stage1(ls):
                  if stage1_done[ls]:
                      return
                  stage1_done[ls] = True
                  emit_wqk()
                  emit_chunk(ls // 2)
                  lp, half = ls // 2, ls % 2
                  hs = slice(half * 512, (half + 1) * 512)
                  sl = slice(ls * 512, (ls + 1) * 512)
                  ps1 = pps.tile([P, 512], f32, name="ps1", tag="pp")
                  for ct in range(CT):
                      nc.tensor.matmul(
                          ps1[:],
                          wqk_sb[:, ct, :],
                          xs[lp][ct][:, hs],
                          start=(ct == 0),
                          stop=(ct == CT - 1),
                      )
                  nc.vector.tensor_copy(qkT[:, sl], ps1[:])
                  nc.sync.dma_start(kqT[:D, sl], qkT[D:, sl])
                  nc.sync.dma_start(kqT[D:, sl], qkT[:D, sl])

              def emit_v_pair(g):
                  emit_wv()
                  for lt in (2 * g, 2 * g + 1):
                      lp2, off = lt // 8, (lt % 8) * P
                      ps2 = pps.tile([P, 512], f32, name="ps2", tag="pp")
                      for ct in range(CT):
                          nc.tensor.matmul(
                              ps2[:, :D],
                              xs[lp2][ct][:, off : off + P],
                              wv_sb[:, ct, :],
                              start=(ct == 0),
                              stop=(ct == CT - 1),
                          )
                      nc.vector.tensor_copy(v_sb[:, lt, :D], ps2[:, :D])

              items = [(isl, g) for isl in range(NSL) for g in range(NPAIR)]
              sc = {}
              pre_e = {}

              def emit_scores(k):
                  isl, g = items[k]
                  if isl == 0:
                      emit_stage1(0)
                      emit_stage1(g // 2)
                  isx = slice(isl * 512, (isl + 1) * 512)
                  jA, jB = 2 * g, 2 * g + 1
                  stp = stps.tile([P, 1024], f32, name="stp", tag="st")
                  if "st" not in ablate:
                      nc.tensor.matmul(
                          stp[:, :512],
                          kqT[:D, jA * P : (jA + 1) * P],
                          qkT[:D, isx],
                          start=True,
                          stop=True,
                      )
                      nc.tensor.matmul(
                          stp[:, 512:],
                          qkT[D:, jB * P : (jB + 1) * P],
                          kqT[D:, isx],
                          start=True,
                          stop=True,
                      )
                  sc[k] = stp

              def emit_exp_chain(k):
                  # DVE exp2 chain for a dve-group item; placement in the DVE
                  # FIFO is tuned via dve_lead (items ahead of consumption)
                  if k in pre_e or k >= len(items) or "exp" in ablate:
                      return
                  if items[k][1] not in dve_groups:
                      return
                  stp = sc[k]
                  e0i = e0p.tile([P, 1024], i32, name="e0i", tag="e0i")
                  nc.vector._custom_dve(
                      _exp_ops["EXP2_INT_ANT"], out=e0i[:], in0=stp[:],
                      s0=_EXP_SCALE, s1=_EXP_BIAS,
                  )
                  e = e0p.tile([P, 1024], bf16, name="ec", tag="ec")
                  nc.vector._custom_dve(
                      _exp_ops["EXP2_CORR_ANT"], out=e[:],
                      in0=e0i[:].bitcast(f32), in1=cst_sb[:],
                      s0=_EXP_C0, s1=_EXP_C1, imm2=_EXP_C2,
                  )
                  pre_e[k] = e

              pvs = {}

              def emit_pv(k, e):
                  isl, g = items[k]
                  if g == 0:
                      pvs[isl] = pvps.tile([D + 1, 1024], f32, name="pvp", tag="pv")
                  if isl == 0:
                      emit_v_pair(g)
                  if "pv" in ablate:
                      return
                  pvp = pvs[isl]
                  jA, jB = 2 * g, 2 * g + 1
                  nc.tensor.matmul(
                      pvp[:, :512],
                      v_sb[:, jA, :],
                      e[:, :512],
                      start=(g == 0),
                      stop=(g == NPAIR - 1),
                  )
                  nc.tensor.matmul(
                      pvp[:, 512:],
                      v_sb[:, jB, :],
                      e[:, 512:],
                      start=(g == 0),
                      stop=(g == NPAIR - 1),
                  )

              def emit_tail(isl):
                  if "tail" in ablate:
                      return
                  isx = slice(isl * 512, (isl + 1) * 512)
                  pvp = pvs[isl]
                  # two bulk copies free the PSUM banks ASAP (next slice's
                  # PV accumulation WARs on them); math continues from SBUF
                  a0 = aop.tile([D + 1, 512], f32, name="a0", tag="a0")
                  a1 = aop.tile([D + 1, 512], f32, name="a1", tag="a1")
                  nc.vector.tensor_copy(a0[:], pvp[:, :512])
                  nc.vector.tensor_copy(a1[:], pvp[:, 512:])
                  nc.vector.tensor_add(ao_all[:, isx], a0[:D], a1[:D])
                  rec_row = rowp.tile([1, 512], f32, name="rec_row", tag="rr")
                  nc.vector.tensor_add(rec_row[:], a0[D : D + 1], a1[D : D + 1])
                  nc.vector.reciprocal_approx_fast(rec_row[:], rec_row[:])
                  dr = drsp.tile([512], f32, name="dr", tag="dr")
                  nc.sync.dma_start(dr[:], rec_row[:])
                  nc.sync.dma_start(
                      rec_all[:, isl * 4 : (isl + 1) * 4],
                      dr.rearrange("(t p) -> p t", p=P),
                  )

              def emit_proj(t, on_scalar=False):
                  if "tail" in ablate or "proj" in ablate:
                      return
                  emit_wo()
                  pp = pps.tile([P, 512], f32, name="ppj", tag="pp")
                  nc.tensor.matmul(
                      pp[:],
                      ao_all[:, t * P : (t + 1) * P],
                      wo_sb[:],
                      start=True,
                      stop=True,
                  )
                  yt = yp.tile([P, C], f32, name="yt", tag="y")
                  if on_scalar:
                      # last slice: ScalarE is idle, DVE chain is critical
                      nc.scalar.activation(
                          yt[:], pp[:], Copy, scale=rec_all[:, t : t + 1]
                      )
                  else:
                      nc.vector.tensor_scalar_mul(yt[:], pp[:],
                                                  rec_all[:, t : t + 1])
                  if "ydma" not in ablate:
                      nc.sync.dma_start(y_d[t * P : (t + 1) * P, :], yt[:])

              # ---- the pipelined stream
              deferred = {}      # item index -> list of thunks (proj work)
              pv_deferred = {}   # item index -> list of thunks (dve-group PVs)
              emit_scores(0)
              emit_scores(1)
              for k in range(len(items)):
                  isl, g = items[k]
                  if isl == 0:
                      # prefetch xt chunks + weights ahead of stage-1/2 use
                      emit_wv()
                      emit_chunk(min(g // 2 + 2, LPC - 1))
                  if k in pre_e:
                      e = pre_e.pop(k)
                      sc.pop(k)
                  else:
                      stp = sc.pop(k)
                      e = expp.tile([P, 1024], bf16, name="e", tag="e")
                      if "exp" not in ablate:
                          # scores are log2-domain (log2e folded into wq)
                          nc.scalar.activation(e[:], stp[:], Exp, scale=LN2)
                  if k + 2 < len(items):
                      emit_scores(k + 2)
                  emit_exp_chain(k + dve_lead)
                  # PVs whose e comes from the DVE chain are deferred so they
                  # never head-of-line-block later score matmuls in the PE FIFO
                  for th in pv_deferred.pop(k, ()):
                      th()
                  if g in dve_groups and g < NPAIR - 1 and k + 2 < len(items):
                      pv_deferred.setdefault(
                          min(k + 2, len(items) - 1), []
                      ).append(lambda kk=k, ee=e: emit_pv(kk, ee))
                  else:
                      emit_pv(k, e)
                  for th in deferred.pop(k, ()):
                      th()
                  if g == NPAIR - 1:
                      emit_tail(isl)
                      # spread the 4 out-projections over later items
                      last = isl == NSL - 1
                      for j in range(4):
                          deferred.setdefault(
                              min(k + proj_defer + j, len(items) - 1), []
                          ).append(
                              lambda t=4 * isl + j, s=last: emit_proj(t, on_scalar=s)
                          )
              # flush any deferral scheduled past the end
              for k in sorted(deferred):
                  for th in deferred[k]:
                      th()

            if reps > 1:
                unroll = 2 if reps % 2 == 0 else 1
                with tc.For_i(0, reps // unroll, 1):
                    for _ in range(unroll):
                        one_body()
            else:
                one_body()

    nc.compile()
    return nc


_nc_cache = {}


def _get_nc(**kw):
    key = tuple(sorted(kw.items()))
    if key not in _nc_cache:
        _nc_cache[key] = build_nc(**kw)
    return _nc_cache[key]


def make_in_maps(x, w_qkv, w_out):
    """Host-side sharding: per-head weight slices, shared transposed input."""
    x = np.asarray(x, dtype=np.float32)
    w_qkv = np.asarray(w_qkv, dtype=np.float32)
    w_out = np.asarray(w_out, dtype=np.float32)
    scale = float(D) ** -0.5
    xt = np.ascontiguousarray(x[0].T).astype(_BF16)  # [C, L]
    xtc = np.ascontiguousarray(xt.reshape(C, 4, 1024).transpose(1, 0, 2))
    in_maps = []
    log2e = float(np.log2(np.e))
    cst = np.full((P, 1), 0x007FFFFF, dtype=np.uint32).view(np.float32)
    for h in range(N_CORES):
        sl = slice(h * D, (h + 1) * D)
        wq = (w_qkv[0 * C :][sl, :] * (scale * log2e)).T  # [C, D], log2 domain
        wk = w_qkv[1 * C :][sl, :].T
        wqk = np.ascontiguousarray(np.concatenate([wq, wk], axis=1)).astype(_BF16)
        wv = np.ascontiguousarray(w_qkv[2 * C :][sl, :].T).astype(_BF16)
        wo = np.ascontiguousarray(w_out[:, sl].T).astype(_BF16)
        in_maps.append({"xt": xtc, "wqk": wqk, "wv": wv, "wo": wo, "cst": cst})
    return in_maps


def kernel(x, w_qkv, w_out, b_out):
    from concourse.bass_utils import run_bass_kernel_spmd

    nc = _get_nc()
    in_maps = make_in_maps(x, w_qkv, w_out)
    res = run_bass_kernel_spmd(nc, in_maps, list(range(N_CORES)))
    y = res.results[0]["y"].copy()
    for i in range(1, N_CORES):
        y += res.results[i]["y"]
    y += np.asarray(b_out, dtype=np.float32)
    return y[None]
